# revision 25
# baseline (speedup 1.0000x reference)
"""Trainium2 Bass kernel for nn_KTM_71339406786898 (optimized v3).

Fused dual-input attention block (see reference):
  q = wq@(x2+x3)+bq, k = wk@(x2*x3)+bk           (CQ=16 channels)
  energy[i,j] = q[:,i].k[:,j];  attn = softmax_j
  out{2,3} = v{2,3} @ attn^T;  z{2,3} = gamma*out + x
  h{2,3} = relu(BN(conv3x3(z)));  out = wo@(w2_1@h2 + w3_1@h3 ...)+bo

Sharding: data-parallel over batch B=8 across 8 NeuronCores.

v3 design notes:
 * exp split across ScalarE (true exp -> bf16) and VectorE (Schraudolph:
   i16 = trunc(x*128/ln2 + (16256-6.85)) bitcast bf16; zero-mean ~2%
   sawtooth that cancels under softmax).  Granules of 2 j-tiles
   alternate ACT/DVE so both engines stream concurrently.
 * all parameters packed into two DRAM blobs (one f32r, one bf16) and x
   pre-stacked host-side -> 4 DMA issues total in setup (dma_start issue
   costs ~0.6us each on a sequencer, so issue count matters).
 * q projection contracts [x2;x3;ones] with duplicated wq rows (no xsum
   tensor); v bias rides on the residual tile xr = x + gamma*bv since
   softmax rows sum to 1; v projections are 2-way row-tiled K=32.
 * normalization is software-pipelined into the next chunk with
   accp bufs=2: d-copy + reciprocal_approx_fast + gpsimd broadcast +
   (acc * r) + residual-add, all off the chunk-critical path.
 * conv3x3 in bf16 (tap stacks via 8 sb->sb DMAs/chunk), relu+BN-bias on
   ScalarE, final fused 1x1 bias via a ones-row, output bounce on ACT.
 * PSUM: 4 banks energy double-buffer + 2 acc + 1 conv = 7.
"""

import sys

import ml_dtypes
import numpy as np

for _p in ("/opt/trn_rl_repo", "/root/.axon_site/_ro/trn_rl_repo"):
    if _p not in sys.path:
        sys.path.append(_p)

import concourse.bass as bass
import concourse.mybir as mybir
import concourse.tile as tile
from concourse import bacc
from concourse.bass_utils import run_bass_kernel_spmd

B, C, H, W = 8, 32, 64, 64
CQ = C // 2
HW = H * W
NCORES = 8

IC = 512            # i-chunk (attention query columns per chunk)
NCH = HW // IC      # 8 chunks
JT = 128            # j-tile (attention key rows per tile = partitions)
NJT = HW // JT      # 32 j-tiles
G = 2               # j-tiles per exp granule
NG = NJT // G       # 16 granules per chunk
XSP = 576           # exp column split: ACT does [0:XSP], DVE [XSP:2*IC]
PW = W + 2          # padded conv width (66)
PHW = PW * (H + 2)  # padded conv plane (66*66)
RPC = IC // W       # spatial rows per chunk (8)
SEG = RPC * PW + W  # stack copy length per chunk (592)

A16 = 184.66496     # 128/ln2
B16 = 16249.15      # 16256 - 6.85 (zero-mean log-ratio calibration)

F32 = mybir.dt.float32
F32R = mybir.dt.float32r
BF16 = mybir.dt.bfloat16
I16 = mybir.dt.int16
AF = mybir.ActivationFunctionType
ALU = mybir.AluOpType

# const pack layouts (free-dim column ranges)
PF_COLS = 66    # f32 pack: wq65[0:16] wk33[16:32] wv23[32:64] brelu[64] c23[65]
PB_COLS = 192   # bf16 pack: w2A w2B w3A w3B w23c wab65 (6 x 32)


def build_program():
    nc = bacc.Bacc("TRN2", target_bir_lowering=False, debug=False)

    xbothd = nc.dram_tensor("xboth", [2 * C + 1, HW], F32R,
                            kind="ExternalInput").ap()
    pfd = nc.dram_tensor("pf32", [128, PF_COLS], F32R, kind="ExternalInput").ap()
    pbd = nc.dram_tensor("pb16", [128, PB_COLS], BF16, kind="ExternalInput").ap()
    outd = nc.dram_tensor("out", [C, HW], BF16, kind="ExternalOutput").ap()

    with tile.TileContext(nc) as tc:
        _emit(nc, tc, xbothd, pfd, pbd, outd)
    nc.compile()
    return nc


def _emit(nc, tc, xbothd, pfd, pbd, outd):
    from contextlib import ExitStack

    ctx = ExitStack()
    with ctx:
        consts = ctx.enter_context(tc.tile_pool(name="consts", bufs=1))
        xp = ctx.enter_context(tc.tile_pool(name="xp", bufs=1))
        qk = ctx.enter_context(tc.tile_pool(name="qk", bufs=1))
        vs = ctx.enter_context(tc.tile_pool(name="vs", bufs=1))
        zpool = ctx.enter_context(tc.tile_pool(name="zpool", bufs=1))
        stk = ctx.enter_context(tc.tile_pool(name="stk", bufs=1))
        es = ctx.enter_context(tc.tile_pool(name="es", bufs=3))
        norm = ctx.enter_context(tc.tile_pool(name="norm", bufs=2))
        rsp = ctx.enter_context(tc.tile_pool(name="rsp", bufs=1))
        obp = ctx.enter_context(tc.tile_pool(name="obp", bufs=2))
        psum = ctx.enter_context(tc.tile_pool(name="psum", bufs=2, space="PSUM"))
        wpsp = ctx.enter_context(tc.tile_pool(name="wpsp", bufs=1, space="PSUM"))
        accp = ctx.enter_context(tc.tile_pool(name="accp", bufs=2, space="PSUM"))
        convp = ctx.enter_context(tc.tile_pool(name="convp", bufs=1, space="PSUM"))

        # ---- setup DMA issues across 3 queues ----
        xa = xp.tile([2 * C + 1, HW], F32R, tag="xa")
        nc.sync.dma_start(out=xa[:], in_=xbothd)
        pf = consts.tile([128, PF_COLS], F32R, tag="pf")
        nc.scalar.dma_start(out=pf[:], in_=pfd)
        pb = consts.tile([128, PB_COLS], BF16, tag="pb")
        nc.gpsimd.dma_start(out=pb[:], in_=pbd)
        x3c = xp.tile([C, HW], F32R, tag="x3c")
        nc.sync.dma_start(out=x3c[:], in_=xbothd[C:2 * C, :])

        wq65 = pf[0:2 * C + 1, 0:CQ]
        wk33 = pf[0:C + 1, CQ:2 * CQ]
        wv23 = pf[0:2 * C, 2 * CQ:2 * CQ + C]
        brelu = pf.bitcast(F32)[0:2 * C, 64:65]
        c23 = pf.bitcast(F32)[0:2 * C, 65:66]
        w2A = pb[:, 0:32]
        w2B = pb[:, 32:64]
        w3A = pb[:, 64:96]
        w3B = pb[:, 96:128]
        w23c = pb[0:2 * C, 128:160]
        wab65 = pb[0:2 * C + 1, 160:192]

        wrm = xp.tile([JT, IC], BF16, tag="wrm")
        nc.vector.memset(wrm[:], 0.25)
        xmul = xp.tile([C + 1, HW], F32R, tag="xmul")
        xr = xp.tile([2 * C, HW], BF16, tag="xr")

        # ---- conv z planes + tap stacks (bf16) ----
        zp = zpool.tile([2 * C, PHW], BF16, tag="zp")
        nc.gpsimd.memset(zp[:], 0.0)
        zp3 = zp.rearrange("p (h w) -> p h w", h=H + 2, w=PW)
        stkA2 = stk.tile([JT, PHW], BF16, tag="stkA2")
        stkB2 = stk.tile([JT, PHW], BF16, tag="stkB2")
        stkA3 = stk.tile([JT, PHW], BF16, tag="stkA3")
        stkB3 = stk.tile([JT, PHW], BF16, tag="stkB3")
        s3 = {nm: t.rearrange("p (h w) -> p h w", h=H + 2, w=PW)
              for nm, t in (("A2", stkA2), ("B2", stkB2),
                            ("A3", stkA3), ("B3", stkB3))}

        vstack = vs.tile([JT, NJT, 2 * C + 1], BF16, tag="vstack")
        nc.gpsimd.memset(vstack[:, :, 2 * C:2 * C + 1], 1.0)

        # ---- q projection (ACT casts; replicas for 4-way row tiling) ----
        q_sb = qk.tile([112, HW], BF16, tag="q")
        k_sb = qk.tile([112, HW], BF16, tag="k")
        QKC = 2 * IC
        for ci in range(4):
            off = ci * QKC
            p = psum.tile([JT, QKC], F32, tag="big", name=f"qp{ci}")
            for s in range(0, QKC, IC):
                nc.tensor.matmul(
                    p[0:CQ, s:s + IC], wq65,
                    xa[:, off + s:off + s + IC], start=True, stop=True)
            nc.scalar.copy(out=q_sb[0:CQ, off:off + QKC], in_=p[0:CQ, :])
            for rg in (1, 2, 3):
                nc.sync.dma_start(out=q_sb[32 * rg:32 * rg + CQ, off:off + QKC],
                                  in_=q_sb[0:CQ, off:off + QKC])

        # ---- xmul = x2*x3 (DVE + gpsimd split), ones row via DMA ----
        XSPL = 2816
        nc.vector.tensor_mul(xmul[0:C, 0:XSPL], xa[0:C, 0:XSPL],
                             x3c[0:C, 0:XSPL])
        nc.gpsimd.tensor_tensor(xmul[0:C, XSPL:HW], xa[0:C, XSPL:HW],
                                x3c[0:C, XSPL:HW], op=ALU.mult)
        nc.scalar.dma_start(out=xmul[C:C + 1, :], in_=xa[2 * C:2 * C + 1, :])

        # ---- xr = x + gamma*bv (residual with v-bias folded in), bf16 ----
        nc.scalar.activation(xr[:], xa[0:2 * C, :], AF.Identity,
                             bias=c23)

        # ---- k projection (DVE casts) ----
        for ci in range(4):
            off = ci * QKC
            p = psum.tile([JT, QKC], F32, tag="big", name=f"kp{ci}")
            for s in range(0, QKC, IC):
                nc.tensor.matmul(
                    p[0:CQ, s:s + IC], wk33,
                    xmul[:, off + s:off + s + IC], start=True, stop=True)
            nc.vector.tensor_copy(out=k_sb[0:CQ, off:off + QKC], in_=p[0:CQ, :])
            for rg in (1, 2, 3):
                nc.scalar.dma_start(out=k_sb[32 * rg:32 * rg + CQ, off:off + QKC],
                                    in_=k_sb[0:CQ, off:off + QKC])

        # ---- v projections: 2-way row-tiled K=32, 4 j-tiles per batch
        # (before k: does not depend on xmul, keeps the PE gap-free) ----
        for b4 in range(NJT // 4):
            vp = psum.tile([JT, 4 * 2 * C], F32, tag="big", name=f"vp{b4}")
            for t in range(4):
                jt = 4 * b4 + t
                js = slice(jt * JT, (jt + 1) * JT)
                nc.tensor.matmul(
                    vp[:, t * 2 * C:t * 2 * C + C],
                    xa[0:C, js], wv23[0:C, :],
                    start=True, stop=True, tile_position=(0, 0))
                nc.tensor.matmul(
                    vp[:, t * 2 * C + C:(t + 1) * 2 * C],
                    xa[C:2 * C, js], wv23[C:2 * C, :],
                    start=True, stop=True, tile_position=(32, 0))
            nc.vector.tensor_copy(
                out=vstack[:, 4 * b4:4 * b4 + 4, 0:2 * C], in_=vp[:])

        # rstk double buffers with preset ones row (final 1x1 bias)
        rstkA = rsp.tile([2 * C + 1, IC], BF16, tag="rstkA")
        nc.vector.memset(rstkA[2 * C:2 * C + 1, :], 1.0)
        rstkB = rsp.tile([2 * C + 1, IC], BF16, tag="rstkB")
        nc.vector.memset(rstkB[2 * C:2 * C + 1, :], 1.0)

        st = {}
        st[0, "rstk"] = rstkA
        st[1, "rstk"] = rstkB

        def emit_recip(ic):
            acc = st[ic, "acc"]
            d_t = norm.tile([1, IC], F32, tag="d", name=f"d{ic}")
            nc.vector.tensor_copy(out=d_t[:], in_=acc[2 * C:2 * C + 1, :])
            r_t = norm.tile([1, IC], F32, tag="r", name=f"r{ic}")
            nc.vector.reciprocal_approx_fast(out=r_t[:], in_=d_t[:])
            st[ic, "r"] = r_t

        def emit_bcast(ic):
            rbc = norm.tile([2 * C, IC], F32, tag="rbc", name=f"rbc{ic}")
            nc.gpsimd.partition_broadcast(rbc[:], st[ic, "r"][:])
            st[ic, "rbc"] = rbc

        def emit_mul(ic):
            zt = norm.tile([2 * C, IC], BF16, tag="zt", name=f"zt{ic}")
            nc.vector.tensor_mul(zt[:], st[ic, "acc"][0:2 * C, :],
                                 st[ic, "rbc"][:])
            st[ic, "zt"] = zt

        def emit_add(ic):
            r0 = RPC * ic
            i0 = ic * IC
            nc.vector.tensor_add(
                zp3[:, 1 + r0:1 + r0 + RPC, 1:1 + W],
                st[ic, "zt"][:].rearrange("p (a b) -> p a b", a=RPC, b=W),
                xr[:, i0:i0 + IC].rearrange("p (a b) -> p a b", a=RPC, b=W),
            )

        def emit_stacks(n, half):
            p0 = PW * RPC * n
            ln = min(SEG, PHW - p0 - 2 * PW - 2)
            r0s, stA, stB = ((0, stkA2, stkB2), (C, stkA3, stkB3))[half]
            for a in range(4):
                offA = (a // 3) * PW + (a % 3)
                nc.sync.dma_start(
                    out=stA[32 * a:32 * a + C, p0:p0 + ln],
                    in_=zp[r0s:r0s + C, p0 + offA:p0 + offA + ln])
                tb = a + 4
                offB = (tb // 3) * PW + (tb % 3)
                nc.gpsimd.dma_start(
                    out=stB[32 * a:32 * a + C, p0:p0 + ln],
                    in_=zp[r0s:r0s + C, p0 + offB:p0 + offB + ln])

        def emit_conv(n):
            r0 = RPC * n
            cp = convp.tile([2 * C, IC], F32, tag="cv", name=f"cp{n}")
            nc.tensor.matmul(cp[0:C, :], w2A, s3["A2"][:, r0:r0 + RPC, 0:W],
                             start=True, stop=False)
            nc.tensor.matmul(cp[0:C, :], w2B, s3["B2"][:, r0:r0 + RPC, 0:W],
                             start=False, stop=False)
            nc.tensor.matmul(cp[0:C, :], w23c[0:C, :],
                             zp3[0:C, 2 + r0:2 + r0 + RPC, 2:2 + W],
                             start=False, stop=True)
            nc.tensor.matmul(cp[C:2 * C, :], w3A,
                             s3["A3"][:, r0:r0 + RPC, 0:W],
                             start=True, stop=False, tile_position=(0, 32))
            nc.tensor.matmul(cp[C:2 * C, :], w3B,
                             s3["B3"][:, r0:r0 + RPC, 0:W],
                             start=False, stop=False, tile_position=(0, 32))
            nc.tensor.matmul(cp[C:2 * C, :], w23c[C:2 * C, :],
                             zp3[C:2 * C, 2 + r0:2 + r0 + RPC, 2:2 + W],
                             start=False, stop=True, tile_position=(32, 32))
            st[n, "cp"] = cp

        def emit_rstk(n):
            rs = st[n % 2, "rstk"]
            nc.scalar.activation(rs[0:2 * C, :], st[n, "cp"][:], AF.Relu,
                                 bias=brelu)

        def emit_wab(n):
            op = convp.tile([2 * C, IC], F32, tag="cv", name=f"op{n}")
            nc.tensor.matmul(op[0:C, :], wab65, st[n % 2, "rstk"][:],
                             start=True, stop=True)
            st[n, "op"] = op

        def emit_ob(n):
            ob = obp.tile([C, IC], BF16, tag="ob", name=f"ob{n}")
            nc.scalar.copy(out=ob[:], in_=st[n, "op"][0:C, :])
            nc.sync.dma_start(out=outd[:, n * IC:(n + 1) * IC], in_=ob[:])

        def emit_E(ic, g):
            i0 = ic * IC
            e_ps = psum.tile([JT, G * IC], F32, tag="big", name=f"eps{ic}_{g}")
            for t in range(G):
                jt = G * g + t
                rt = jt % 4
                nc.tensor.matmul(
                    e_ps[:, t * IC:(t + 1) * IC],
                    k_sb[32 * rt:32 * rt + CQ, jt * JT:(jt + 1) * JT],
                    q_sb[32 * rt:32 * rt + CQ, i0:i0 + IC],
                    start=True, stop=True, tile_position=(32 * rt, 0))
            st[ic, g, "eps"] = e_ps

        def emit_exp(ic, g):
            e_ps = st.pop((ic, g, "eps"))
            e_sb = es.tile([JT, G * IC], BF16, tag="esb", name=f"esb{ic}_{g}")
            nc.scalar.activation(e_sb[:, 0:XSP], e_ps[:, 0:XSP], AF.Exp)
            nc.vector.tensor_scalar(
                out=e_sb.bitcast(I16)[:, XSP:G * IC],
                in0=e_ps[:, XSP:G * IC],
                scalar1=A16, scalar2=B16, op0=ALU.mult, op1=ALU.add)
            st[ic, g, "esb"] = e_sb

        def emit_AV(ic, g):
            if (ic, "acc") not in st:
                st[ic, "acc"] = accp.tile([2 * C + 1, IC], F32, tag="acc",
                                          name=f"acc{ic}")
            acc = st[ic, "acc"]
            e_sb = st.pop((ic, g, "esb"))
            for t in range(G):
                jt = G * g + t
                nc.tensor.matmul(
                    acc[:], vstack[:, jt, :], e_sb[:, t * IC:(t + 1) * IC],
                    start=(jt == 0), stop=(jt == NJT - 1))

        # ---- PE warm-up burst: dense full-array matmuls with one
        # stationary operand flip HAM to K=8/8 (2.4 GHz) ----
        wps = wpsp.tile([JT, IC], F32, tag="wps")
        for i in range(24):
            nc.tensor.matmul(wps[:], wrm[:, 0:JT], wrm[:],
                             start=True, stop=True)

        # ---- main loop: 2-chunk software pipeline over 16 granules ----
        NTOT = NCH * NG
        emit_E(0, 0)
        for gi in range(NTOT):
            ic, g = divmod(gi, NG)
            if gi + 1 < NTOT:
                ic2, g2 = divmod(gi + 1, NG)
                emit_E(ic2, g2)
            emit_exp(ic, g)
            emit_AV(ic, g)
            if g == 0 and ic >= 1:
                emit_recip(ic - 1)
            elif g == 1 and ic >= 1:
                emit_bcast(ic - 1)
            elif g == 2 and ic >= 1:
                emit_mul(ic - 1)
            elif g == 3 and ic >= 1:
                emit_add(ic - 1)
            elif g == 4 and ic >= 2:
                emit_stacks(ic - 2, 0)
            elif g == 5 and ic >= 2:
                emit_stacks(ic - 2, 1)
            elif g == 6 and ic >= 2:
                emit_conv(ic - 2)
            elif g == 8 and ic >= 2:
                emit_rstk(ic - 2)
            elif g == 10 and ic >= 2:
                emit_wab(ic - 2)
            elif g == 12 and ic >= 2:
                emit_ob(ic - 2)

        # ---- drain ----
        ic = NCH - 1
        emit_recip(ic)
        emit_bcast(ic)
        emit_mul(ic)
        emit_add(ic)
        for n in (NCH - 2, NCH - 1):
            emit_stacks(n, 0)
            emit_stacks(n, 1)
            emit_conv(n)
            emit_rstk(n)
            emit_wab(n)
            emit_ob(n)


def prepare_params(wq, bq, wk, bk, wv2, bv2, wv3, bv3, gamma2, gamma3,
                   w2_3, bn2_s, bn2_b, w2_1, b2_1,
                   w3_3, bn3_s, bn3_b, w3_1, b3_1, wo, bo):
    f = np.float32
    bf = ml_dtypes.bfloat16
    wq, bq, wk, bk = (np.asarray(a, f) for a in (wq, bq, wk, bk))
    wv2, bv2, wv3, bv3 = (np.asarray(a, f) for a in (wv2, bv2, wv3, bv3))
    w2_3, bn2_s, bn2_b = (np.asarray(a, f) for a in (w2_3, bn2_s, bn2_b))
    w3_3, bn3_s, bn3_b = (np.asarray(a, f) for a in (w3_3, bn3_s, bn3_b))
    w2_1, b2_1, w3_1, b3_1 = (np.asarray(a, f) for a in (w2_1, b2_1, w3_1, b3_1))
    wo, bo = np.asarray(wo, f), np.asarray(bo, f)
    g2 = f(np.asarray(gamma2).reshape(-1)[0])
    g3 = f(np.asarray(gamma3).reshape(-1)[0])

    pf32 = np.zeros((128, PF_COLS), f)
    pf32[0:C, 0:CQ] = wq.T
    pf32[C:2 * C, 0:CQ] = wq.T
    pf32[2 * C, 0:CQ] = bq
    pf32[0:C, CQ:2 * CQ] = wk.T
    pf32[C, CQ:2 * CQ] = bk
    pf32[0:C, 2 * CQ:2 * CQ + C] = wv2.T * g2
    pf32[C:2 * C, 2 * CQ:2 * CQ + C] = wv3.T * g3
    pf32[0:C, 64] = bn2_b
    pf32[C:2 * C, 64] = bn3_b
    pf32[0:C, 65] = g2 * bv2
    pf32[C:2 * C, 65] = g3 * bv3

    def conv_stacks(w3x3, bn_s):
        ws = w3x3 * bn_s[:, None, None, None]
        Am = np.zeros((4 * C, C), f)
        Bm = np.zeros((4 * C, C), f)
        for a in range(4):
            Am[32 * a:32 * a + C] = ws[:, :, a // 3, a % 3].T
            tb = a + 4
            Bm[32 * a:32 * a + C] = ws[:, :, tb // 3, tb % 3].T
        return Am, Bm, ws[:, :, 2, 2].T.copy()

    w2A, w2B, w2c = conv_stacks(w2_3, bn2_s)
    w3A, w3B, w3c = conv_stacks(w3_3, bn3_s)

    pb16 = np.zeros((128, PB_COLS), f)
    pb16[:, 0:32] = w2A
    pb16[:, 32:64] = w2B
    pb16[:, 64:96] = w3A
    pb16[:, 96:128] = w3B
    pb16[0:C, 128:160] = w2c
    pb16[C:2 * C, 128:160] = w3c
    pb16[0:C, 160:192] = (wo @ w2_1).T
    pb16[C:2 * C, 160:192] = (wo @ w3_1).T
    pb16[2 * C, 160:192] = wo @ (b2_1 + b3_1) + bo

    return {"pf32": pf32, "pb16": pb16.astype(bf)}


_CACHED = {}


def _get_program():
    if "nc" not in _CACHED:
        _CACHED["nc"] = build_program()
    return _CACHED["nc"]


def make_in_maps(x2, x3, params):
    x2 = np.asarray(x2, np.float32).reshape(B, C, HW)
    x3 = np.asarray(x3, np.float32).reshape(B, C, HW)
    ones = np.ones((1, HW), np.float32)
    return [
        {"xboth": np.concatenate([x2[b], x3[b], ones], 0), **params}
        for b in range(NCORES)
    ]


def kernel(x2, x3, **kw):
    params = prepare_params(**kw)
    nc = _get_program()
    in_maps = make_in_maps(x2, x3, params)
    res = run_bass_kernel_spmd(nc, in_maps, list(range(NCORES)))
    out = np.stack([np.asarray(res.results[b]["out"]).astype(np.float32)
                    .reshape(C, H, W) for b in range(NCORES)])
    return out


def _ensure_ntff_hook():
    """Register the ctypes NTFF profile hook (agent image lacks axon_hooks)."""
    import contextlib
    import ctypes
    import types

    if "antenv.axon_hooks" in sys.modules:
        return
    so_path = "/opt/axon/libaxon_pjrt.so"
    lib = ctypes.CDLL(so_path)
    lib.axon_start_nrt_profile.argtypes = [
        ctypes.POINTER(ctypes.c_int64), ctypes.c_size_t]
    lib.axon_start_nrt_profile.restype = ctypes.c_int64
    lib.axon_stop_nrt_profile.argtypes = [ctypes.c_char_p]
    lib.axon_stop_nrt_profile.restype = ctypes.c_int64

    @contextlib.contextmanager
    def _hook(output_dir, device_ids):
        import jax
        jax.devices()
        if device_ids:
            ids = (ctypes.c_int64 * len(device_ids))(*device_ids)
            rc = lib.axon_start_nrt_profile(ids, len(device_ids))
        else:
            rc = lib.axon_start_nrt_profile(None, 0)
        if rc != 0:
            raise RuntimeError(f"axon_start_nrt_profile rc={rc}")
        try:
            yield
        finally:
            n = lib.axon_stop_nrt_profile(str(output_dir).encode())
            if n < 0:
                raise RuntimeError(f"axon_stop_nrt_profile rc={n}")
            if n == 0:
                print("WARNING: NTFF capture wrote 0 files")

    mod = types.ModuleType("antenv.axon_hooks")
    mod.get_axon_ntff_profile_hook = lambda: _hook
    mod.set_axon_ntff_profile_hook = lambda h: None
    sys.modules["antenv.axon_hooks"] = mod


def run_traced(x2, x3, trace_cores=None, **kw):
    _ensure_ntff_hook()
    params = prepare_params(**kw)
    nc = _get_program()
    in_maps = make_in_maps(x2, x3, params)
    res = run_bass_kernel_spmd(nc, in_maps, list(range(NCORES)),
                               trace=True, trace_cores=trace_cores)
    out = np.stack([np.asarray(res.results[b]["out"]).astype(np.float32)
                    .reshape(C, H, W) for b in range(NCORES)])
    return out, res


# revision 26
# speedup vs baseline: 1.0036x; 1.0036x over previous
"""Trainium2 Bass kernel for nn_KTM_71339406786898 (optimized v3).

Fused dual-input attention block (see reference):
  q = wq@(x2+x3)+bq, k = wk@(x2*x3)+bk           (CQ=16 channels)
  energy[i,j] = q[:,i].k[:,j];  attn = softmax_j
  out{2,3} = v{2,3} @ attn^T;  z{2,3} = gamma*out + x
  h{2,3} = relu(BN(conv3x3(z)));  out = wo@(w2_1@h2 + w3_1@h3 ...)+bo

Sharding: data-parallel over batch B=8 across 8 NeuronCores.

v3 design notes:
 * exp split across ScalarE (true exp -> bf16) and VectorE (Schraudolph:
   i16 = trunc(x*128/ln2 + (16256-6.85)) bitcast bf16; zero-mean ~2%
   sawtooth that cancels under softmax).  Granules of 2 j-tiles
   alternate ACT/DVE so both engines stream concurrently.
 * all parameters packed into two DRAM blobs (one f32r, one bf16) and x
   pre-stacked host-side -> 4 DMA issues total in setup (dma_start issue
   costs ~0.6us each on a sequencer, so issue count matters).
 * q projection contracts [x2;x3;ones] with duplicated wq rows (no xsum
   tensor); v bias rides on the residual tile xr = x + gamma*bv since
   softmax rows sum to 1; v projections are 2-way row-tiled K=32.
 * normalization is software-pipelined into the next chunk with
   accp bufs=2: d-copy + reciprocal_approx_fast + gpsimd broadcast +
   (acc * r) + residual-add, all off the chunk-critical path.
 * conv3x3 in bf16 (tap stacks via 8 sb->sb DMAs/chunk), relu+BN-bias on
   ScalarE, final fused 1x1 bias via a ones-row, output bounce on ACT.
 * PSUM: 4 banks energy double-buffer + 2 acc + 1 conv = 7.
"""

import sys

import ml_dtypes
import numpy as np

for _p in ("/opt/trn_rl_repo", "/root/.axon_site/_ro/trn_rl_repo"):
    if _p not in sys.path:
        sys.path.append(_p)

import concourse.bass as bass
import concourse.mybir as mybir
import concourse.tile as tile
from concourse import bacc
from concourse.bass_utils import run_bass_kernel_spmd

B, C, H, W = 8, 32, 64, 64
CQ = C // 2
HW = H * W
NCORES = 8

IC = 512            # i-chunk (attention query columns per chunk)
NCH = HW // IC      # 8 chunks
JT = 128            # j-tile (attention key rows per tile = partitions)
NJT = HW // JT      # 32 j-tiles
G = 2               # j-tiles per exp granule
NG = NJT // G       # 16 granules per chunk
XSP = 576           # exp column split: ACT does [0:XSP], DVE [XSP:2*IC]
PW = W + 2          # padded conv width (66)
PHW = PW * (H + 2)  # padded conv plane (66*66)
RPC = IC // W       # spatial rows per chunk (8)
SEG = RPC * PW + W  # stack copy length per chunk (592)

A16 = 184.66496     # 128/ln2
B16 = 16249.15      # 16256 - 6.85 (zero-mean log-ratio calibration)

F32 = mybir.dt.float32
F32R = mybir.dt.float32r
BF16 = mybir.dt.bfloat16
I16 = mybir.dt.int16
AF = mybir.ActivationFunctionType
ALU = mybir.AluOpType

# const pack layouts (free-dim column ranges)
PF_COLS = 66    # f32 pack: wq65[0:16] wk33[16:32] wv23[32:64] brelu[64] c23[65]
PB_COLS = 192   # bf16 pack: w2A w2B w3A w3B w23c wab65 (6 x 32)


def build_program():
    nc = bacc.Bacc("TRN2", target_bir_lowering=False, debug=False)

    xbothd = nc.dram_tensor("xboth", [2 * C + 1, HW], F32R,
                            kind="ExternalInput").ap()
    pfd = nc.dram_tensor("pf32", [128, PF_COLS], F32R, kind="ExternalInput").ap()
    pbd = nc.dram_tensor("pb16", [128, PB_COLS], BF16, kind="ExternalInput").ap()
    outd = nc.dram_tensor("out", [C, HW], BF16, kind="ExternalOutput").ap()

    with tile.TileContext(nc) as tc:
        _emit(nc, tc, xbothd, pfd, pbd, outd)
    nc.compile()
    return nc


def _emit(nc, tc, xbothd, pfd, pbd, outd):
    from contextlib import ExitStack

    ctx = ExitStack()
    with ctx:
        consts = ctx.enter_context(tc.tile_pool(name="consts", bufs=1))
        xp = ctx.enter_context(tc.tile_pool(name="xp", bufs=1))
        qk = ctx.enter_context(tc.tile_pool(name="qk", bufs=1))
        vs = ctx.enter_context(tc.tile_pool(name="vs", bufs=1))
        zpool = ctx.enter_context(tc.tile_pool(name="zpool", bufs=1))
        stk = ctx.enter_context(tc.tile_pool(name="stk", bufs=1))
        es = ctx.enter_context(tc.tile_pool(name="es", bufs=3))
        norm = ctx.enter_context(tc.tile_pool(name="norm", bufs=2))
        rsp = ctx.enter_context(tc.tile_pool(name="rsp", bufs=1))
        obp = ctx.enter_context(tc.tile_pool(name="obp", bufs=2))
        psum = ctx.enter_context(tc.tile_pool(name="psum", bufs=2, space="PSUM"))
        wpsp = ctx.enter_context(tc.tile_pool(name="wpsp", bufs=1, space="PSUM"))
        accp = ctx.enter_context(tc.tile_pool(name="accp", bufs=2, space="PSUM"))
        convp = ctx.enter_context(tc.tile_pool(name="convp", bufs=1, space="PSUM"))

        # ---- setup DMA issues across 3 queues ----
        xa = xp.tile([2 * C + 1, HW], F32R, tag="xa")
        nc.sync.dma_start(out=xa[:], in_=xbothd)
        pf = consts.tile([128, PF_COLS], F32R, tag="pf")
        nc.scalar.dma_start(out=pf[:], in_=pfd)
        pb = consts.tile([128, PB_COLS], BF16, tag="pb")
        nc.gpsimd.dma_start(out=pb[:], in_=pbd)
        x3c = xp.tile([C, HW], F32R, tag="x3c")
        nc.sync.dma_start(out=x3c[:], in_=xbothd[C:2 * C, :])

        wq65 = pf[0:2 * C + 1, 0:CQ]
        wk33 = pf[0:C + 1, CQ:2 * CQ]
        wv23 = pf[0:2 * C, 2 * CQ:2 * CQ + C]
        brelu = pf.bitcast(F32)[0:2 * C, 64:65]
        c23 = pf.bitcast(F32)[0:2 * C, 65:66]
        w2A = pb[:, 0:32]
        w2B = pb[:, 32:64]
        w3A = pb[:, 64:96]
        w3B = pb[:, 96:128]
        w23c = pb[0:2 * C, 128:160]
        wab65 = pb[0:2 * C + 1, 160:192]

        wrm = xp.tile([JT, IC], BF16, tag="wrm")
        nc.vector.memset(wrm[:], 0.25)
        xmul = xp.tile([C + 1, HW], F32R, tag="xmul")
        xr = xp.tile([2 * C, HW], BF16, tag="xr")

        # ---- conv z planes + tap stacks (bf16) ----
        zp = zpool.tile([2 * C, PHW], BF16, tag="zp")
        nc.gpsimd.memset(zp[:], 0.0)
        zp3 = zp.rearrange("p (h w) -> p h w", h=H + 2, w=PW)
        stkA2 = stk.tile([JT, PHW], BF16, tag="stkA2")
        stkB2 = stk.tile([JT, PHW], BF16, tag="stkB2")
        stkA3 = stk.tile([JT, PHW], BF16, tag="stkA3")
        stkB3 = stk.tile([JT, PHW], BF16, tag="stkB3")
        s3 = {nm: t.rearrange("p (h w) -> p h w", h=H + 2, w=PW)
              for nm, t in (("A2", stkA2), ("B2", stkB2),
                            ("A3", stkA3), ("B3", stkB3))}

        vstack = vs.tile([JT, NJT, 2 * C + 1], BF16, tag="vstack")
        nc.gpsimd.memset(vstack[:, :, 2 * C:2 * C + 1], 1.0)

        # ---- q projection (ACT casts; replicas for 4-way row tiling) ----
        q_sb = qk.tile([112, HW], BF16, tag="q")
        k_sb = qk.tile([112, HW], BF16, tag="k")
        QKC = 2 * IC
        for ci in range(4):
            off = ci * QKC
            p = psum.tile([JT, QKC], F32, tag="big", name=f"qp{ci}")
            for s in range(0, QKC, IC):
                nc.tensor.matmul(
                    p[0:CQ, s:s + IC], wq65,
                    xa[:, off + s:off + s + IC], start=True, stop=True)
            if ci < 2:
                nc.scalar.copy(out=q_sb[0:CQ, off:off + QKC], in_=p[0:CQ, :])
            else:
                nc.vector.tensor_copy(out=q_sb[0:CQ, off:off + QKC],
                                      in_=p[0:CQ, :])
            for rg in (1, 2, 3):
                nc.sync.dma_start(out=q_sb[32 * rg:32 * rg + CQ, off:off + QKC],
                                  in_=q_sb[0:CQ, off:off + QKC])

        # ---- xmul = x2*x3 (DVE + gpsimd split), ones row via DMA ----
        XSPL = 2816
        nc.vector.tensor_mul(xmul[0:C, 0:XSPL], xa[0:C, 0:XSPL],
                             x3c[0:C, 0:XSPL])
        nc.gpsimd.tensor_tensor(xmul[0:C, XSPL:HW], xa[0:C, XSPL:HW],
                                x3c[0:C, XSPL:HW], op=ALU.mult)
        nc.scalar.dma_start(out=xmul[C:C + 1, :], in_=xa[2 * C:2 * C + 1, :])

        # ---- xr = x + gamma*bv (residual with v-bias folded in), bf16 ----
        nc.scalar.activation(xr[:], xa[0:2 * C, :], AF.Identity,
                             bias=c23)

        # ---- k projection (DVE casts) ----
        for ci in range(4):
            off = ci * QKC
            p = psum.tile([JT, QKC], F32, tag="big", name=f"kp{ci}")
            for s in range(0, QKC, IC):
                nc.tensor.matmul(
                    p[0:CQ, s:s + IC], wk33,
                    xmul[:, off + s:off + s + IC], start=True, stop=True)
            nc.vector.tensor_copy(out=k_sb[0:CQ, off:off + QKC], in_=p[0:CQ, :])
            for rg in (1, 2, 3):
                nc.scalar.dma_start(out=k_sb[32 * rg:32 * rg + CQ, off:off + QKC],
                                    in_=k_sb[0:CQ, off:off + QKC])

        # ---- v projections: 2-way row-tiled K=32, 4 j-tiles per batch
        # (before k: does not depend on xmul, keeps the PE gap-free) ----
        for b4 in range(NJT // 4):
            vp = psum.tile([JT, 4 * 2 * C], F32, tag="big", name=f"vp{b4}")
            for t in range(4):
                jt = 4 * b4 + t
                js = slice(jt * JT, (jt + 1) * JT)
                nc.tensor.matmul(
                    vp[:, t * 2 * C:t * 2 * C + C],
                    xa[0:C, js], wv23[0:C, :],
                    start=True, stop=True, tile_position=(0, 0))
                nc.tensor.matmul(
                    vp[:, t * 2 * C + C:(t + 1) * 2 * C],
                    xa[C:2 * C, js], wv23[C:2 * C, :],
                    start=True, stop=True, tile_position=(32, 0))
            nc.vector.tensor_copy(
                out=vstack[:, 4 * b4:4 * b4 + 4, 0:2 * C], in_=vp[:])

        # rstk double buffers with preset ones row (final 1x1 bias)
        rstkA = rsp.tile([2 * C + 1, IC], BF16, tag="rstkA")
        nc.vector.memset(rstkA[2 * C:2 * C + 1, :], 1.0)
        rstkB = rsp.tile([2 * C + 1, IC], BF16, tag="rstkB")
        nc.vector.memset(rstkB[2 * C:2 * C + 1, :], 1.0)

        st = {}
        st[0, "rstk"] = rstkA
        st[1, "rstk"] = rstkB

        def emit_recip(ic):
            acc = st[ic, "acc"]
            d_t = norm.tile([1, IC], F32, tag="d", name=f"d{ic}")
            nc.vector.tensor_copy(out=d_t[:], in_=acc[2 * C:2 * C + 1, :])
            r_t = norm.tile([1, IC], F32, tag="r", name=f"r{ic}")
            nc.vector.reciprocal_approx_fast(out=r_t[:], in_=d_t[:])
            st[ic, "r"] = r_t

        def emit_bcast(ic):
            rbc = norm.tile([2 * C, IC], F32, tag="rbc", name=f"rbc{ic}")
            nc.gpsimd.partition_broadcast(rbc[:], st[ic, "r"][:])
            st[ic, "rbc"] = rbc

        def emit_mul(ic):
            zt = norm.tile([2 * C, IC], BF16, tag="zt", name=f"zt{ic}")
            nc.vector.tensor_mul(zt[:], st[ic, "acc"][0:2 * C, :],
                                 st[ic, "rbc"][:])
            st[ic, "zt"] = zt

        def emit_add(ic):
            r0 = RPC * ic
            i0 = ic * IC
            nc.vector.tensor_add(
                zp3[:, 1 + r0:1 + r0 + RPC, 1:1 + W],
                st[ic, "zt"][:].rearrange("p (a b) -> p a b", a=RPC, b=W),
                xr[:, i0:i0 + IC].rearrange("p (a b) -> p a b", a=RPC, b=W),
            )

        def emit_stacks(n, half):
            p0 = PW * RPC * n
            ln = min(SEG, PHW - p0 - 2 * PW - 2)
            r0s, stA, stB = ((0, stkA2, stkB2), (C, stkA3, stkB3))[half]
            for a in range(4):
                offA = (a // 3) * PW + (a % 3)
                nc.sync.dma_start(
                    out=stA[32 * a:32 * a + C, p0:p0 + ln],
                    in_=zp[r0s:r0s + C, p0 + offA:p0 + offA + ln])
                tb = a + 4
                offB = (tb // 3) * PW + (tb % 3)
                nc.gpsimd.dma_start(
                    out=stB[32 * a:32 * a + C, p0:p0 + ln],
                    in_=zp[r0s:r0s + C, p0 + offB:p0 + offB + ln])

        def emit_conv(n):
            r0 = RPC * n
            cp = convp.tile([2 * C, IC], F32, tag="cv", name=f"cp{n}")
            nc.tensor.matmul(cp[0:C, :], w2A, s3["A2"][:, r0:r0 + RPC, 0:W],
                             start=True, stop=False)
            nc.tensor.matmul(cp[0:C, :], w2B, s3["B2"][:, r0:r0 + RPC, 0:W],
                             start=False, stop=False)
            nc.tensor.matmul(cp[0:C, :], w23c[0:C, :],
                             zp3[0:C, 2 + r0:2 + r0 + RPC, 2:2 + W],
                             start=False, stop=True)
            nc.tensor.matmul(cp[C:2 * C, :], w3A,
                             s3["A3"][:, r0:r0 + RPC, 0:W],
                             start=True, stop=False, tile_position=(0, 32))
            nc.tensor.matmul(cp[C:2 * C, :], w3B,
                             s3["B3"][:, r0:r0 + RPC, 0:W],
                             start=False, stop=False, tile_position=(0, 32))
            nc.tensor.matmul(cp[C:2 * C, :], w23c[C:2 * C, :],
                             zp3[C:2 * C, 2 + r0:2 + r0 + RPC, 2:2 + W],
                             start=False, stop=True, tile_position=(32, 32))
            st[n, "cp"] = cp

        def emit_rstk(n):
            rs = st[n % 2, "rstk"]
            nc.scalar.activation(rs[0:2 * C, :], st[n, "cp"][:], AF.Relu,
                                 bias=brelu)

        def emit_wab(n):
            op = convp.tile([2 * C, IC], F32, tag="cv", name=f"op{n}")
            nc.tensor.matmul(op[0:C, :], wab65, st[n % 2, "rstk"][:],
                             start=True, stop=True)
            st[n, "op"] = op

        def emit_ob(n):
            ob = obp.tile([C, IC], BF16, tag="ob", name=f"ob{n}")
            nc.scalar.copy(out=ob[:], in_=st[n, "op"][0:C, :])
            nc.sync.dma_start(out=outd[:, n * IC:(n + 1) * IC], in_=ob[:])

        def emit_E(ic, g):
            i0 = ic * IC
            e_ps = psum.tile([JT, G * IC], F32, tag="big", name=f"eps{ic}_{g}")
            for t in range(G):
                jt = G * g + t
                rt = jt % 4
                nc.tensor.matmul(
                    e_ps[:, t * IC:(t + 1) * IC],
                    k_sb[32 * rt:32 * rt + CQ, jt * JT:(jt + 1) * JT],
                    q_sb[32 * rt:32 * rt + CQ, i0:i0 + IC],
                    start=True, stop=True, tile_position=(32 * rt, 0))
            st[ic, g, "eps"] = e_ps

        def emit_exp(ic, g):
            e_ps = st.pop((ic, g, "eps"))
            e_sb = es.tile([JT, G * IC], BF16, tag="esb", name=f"esb{ic}_{g}")
            nc.scalar.activation(e_sb[:, 0:XSP], e_ps[:, 0:XSP], AF.Exp)
            nc.vector.tensor_scalar(
                out=e_sb.bitcast(I16)[:, XSP:G * IC],
                in0=e_ps[:, XSP:G * IC],
                scalar1=A16, scalar2=B16, op0=ALU.mult, op1=ALU.add)
            st[ic, g, "esb"] = e_sb

        def emit_AV(ic, g):
            if (ic, "acc") not in st:
                st[ic, "acc"] = accp.tile([2 * C + 1, IC], F32, tag="acc",
                                          name=f"acc{ic}")
            acc = st[ic, "acc"]
            e_sb = st.pop((ic, g, "esb"))
            for t in range(G):
                jt = G * g + t
                nc.tensor.matmul(
                    acc[:], vstack[:, jt, :], e_sb[:, t * IC:(t + 1) * IC],
                    start=(jt == 0), stop=(jt == NJT - 1))

        # ---- PE warm-up burst: dense full-array matmuls with one
        # stationary operand flip HAM to K=8/8 (2.4 GHz) ----
        wps = wpsp.tile([JT, IC], F32, tag="wps")
        for i in range(40):
            nc.tensor.matmul(wps[:], wrm[:, 0:JT], wrm[:],
                             start=True, stop=True)

        # ---- main loop: 2-chunk software pipeline over 16 granules ----
        NTOT = NCH * NG
        emit_E(0, 0)
        for gi in range(NTOT):
            ic, g = divmod(gi, NG)
            if gi + 1 < NTOT:
                ic2, g2 = divmod(gi + 1, NG)
                emit_E(ic2, g2)
            emit_exp(ic, g)
            emit_AV(ic, g)
            if g == 0 and ic >= 1:
                emit_recip(ic - 1)
            elif g == 1 and ic >= 1:
                emit_bcast(ic - 1)
            elif g == 2 and ic >= 1:
                emit_mul(ic - 1)
            elif g == 3 and ic >= 1:
                emit_add(ic - 1)
            elif g == 4 and ic >= 2:
                emit_stacks(ic - 2, 0)
            elif g == 5 and ic >= 2:
                emit_stacks(ic - 2, 1)
            elif g == 6 and ic >= 2:
                emit_conv(ic - 2)
            elif g == 8 and ic >= 2:
                emit_rstk(ic - 2)
            elif g == 10 and ic >= 2:
                emit_wab(ic - 2)
            elif g == 12 and ic >= 2:
                emit_ob(ic - 2)

        # ---- drain ----
        ic = NCH - 1
        emit_recip(ic)
        emit_bcast(ic)
        emit_mul(ic)
        emit_add(ic)
        for n in (NCH - 2, NCH - 1):
            emit_stacks(n, 0)
            emit_stacks(n, 1)
            emit_conv(n)
            emit_rstk(n)
            emit_wab(n)
            emit_ob(n)


def prepare_params(wq, bq, wk, bk, wv2, bv2, wv3, bv3, gamma2, gamma3,
                   w2_3, bn2_s, bn2_b, w2_1, b2_1,
                   w3_3, bn3_s, bn3_b, w3_1, b3_1, wo, bo):
    f = np.float32
    bf = ml_dtypes.bfloat16
    wq, bq, wk, bk = (np.asarray(a, f) for a in (wq, bq, wk, bk))
    wv2, bv2, wv3, bv3 = (np.asarray(a, f) for a in (wv2, bv2, wv3, bv3))
    w2_3, bn2_s, bn2_b = (np.asarray(a, f) for a in (w2_3, bn2_s, bn2_b))
    w3_3, bn3_s, bn3_b = (np.asarray(a, f) for a in (w3_3, bn3_s, bn3_b))
    w2_1, b2_1, w3_1, b3_1 = (np.asarray(a, f) for a in (w2_1, b2_1, w3_1, b3_1))
    wo, bo = np.asarray(wo, f), np.asarray(bo, f)
    g2 = f(np.asarray(gamma2).reshape(-1)[0])
    g3 = f(np.asarray(gamma3).reshape(-1)[0])

    pf32 = np.zeros((128, PF_COLS), f)
    pf32[0:C, 0:CQ] = wq.T
    pf32[C:2 * C, 0:CQ] = wq.T
    pf32[2 * C, 0:CQ] = bq
    pf32[0:C, CQ:2 * CQ] = wk.T
    pf32[C, CQ:2 * CQ] = bk
    pf32[0:C, 2 * CQ:2 * CQ + C] = wv2.T * g2
    pf32[C:2 * C, 2 * CQ:2 * CQ + C] = wv3.T * g3
    pf32[0:C, 64] = bn2_b
    pf32[C:2 * C, 64] = bn3_b
    pf32[0:C, 65] = g2 * bv2
    pf32[C:2 * C, 65] = g3 * bv3

    def conv_stacks(w3x3, bn_s):
        ws = w3x3 * bn_s[:, None, None, None]
        Am = np.zeros((4 * C, C), f)
        Bm = np.zeros((4 * C, C), f)
        for a in range(4):
            Am[32 * a:32 * a + C] = ws[:, :, a // 3, a % 3].T
            tb = a + 4
            Bm[32 * a:32 * a + C] = ws[:, :, tb // 3, tb % 3].T
        return Am, Bm, ws[:, :, 2, 2].T.copy()

    w2A, w2B, w2c = conv_stacks(w2_3, bn2_s)
    w3A, w3B, w3c = conv_stacks(w3_3, bn3_s)

    pb16 = np.zeros((128, PB_COLS), f)
    pb16[:, 0:32] = w2A
    pb16[:, 32:64] = w2B
    pb16[:, 64:96] = w3A
    pb16[:, 96:128] = w3B
    pb16[0:C, 128:160] = w2c
    pb16[C:2 * C, 128:160] = w3c
    pb16[0:C, 160:192] = (wo @ w2_1).T
    pb16[C:2 * C, 160:192] = (wo @ w3_1).T
    pb16[2 * C, 160:192] = wo @ (b2_1 + b3_1) + bo

    return {"pf32": pf32, "pb16": pb16.astype(bf)}


_CACHED = {}


def _get_program():
    if "nc" not in _CACHED:
        _CACHED["nc"] = build_program()
    return _CACHED["nc"]


def make_in_maps(x2, x3, params):
    x2 = np.asarray(x2, np.float32).reshape(B, C, HW)
    x3 = np.asarray(x3, np.float32).reshape(B, C, HW)
    ones = np.ones((1, HW), np.float32)
    return [
        {"xboth": np.concatenate([x2[b], x3[b], ones], 0), **params}
        for b in range(NCORES)
    ]


def kernel(x2, x3, **kw):
    params = prepare_params(**kw)
    nc = _get_program()
    in_maps = make_in_maps(x2, x3, params)
    res = run_bass_kernel_spmd(nc, in_maps, list(range(NCORES)))
    out = np.stack([np.asarray(res.results[b]["out"]).astype(np.float32)
                    .reshape(C, H, W) for b in range(NCORES)])
    return out


def _ensure_ntff_hook():
    """Register the ctypes NTFF profile hook (agent image lacks axon_hooks)."""
    import contextlib
    import ctypes
    import types

    if "antenv.axon_hooks" in sys.modules:
        return
    so_path = "/opt/axon/libaxon_pjrt.so"
    lib = ctypes.CDLL(so_path)
    lib.axon_start_nrt_profile.argtypes = [
        ctypes.POINTER(ctypes.c_int64), ctypes.c_size_t]
    lib.axon_start_nrt_profile.restype = ctypes.c_int64
    lib.axon_stop_nrt_profile.argtypes = [ctypes.c_char_p]
    lib.axon_stop_nrt_profile.restype = ctypes.c_int64

    @contextlib.contextmanager
    def _hook(output_dir, device_ids):
        import jax
        jax.devices()
        if device_ids:
            ids = (ctypes.c_int64 * len(device_ids))(*device_ids)
            rc = lib.axon_start_nrt_profile(ids, len(device_ids))
        else:
            rc = lib.axon_start_nrt_profile(None, 0)
        if rc != 0:
            raise RuntimeError(f"axon_start_nrt_profile rc={rc}")
        try:
            yield
        finally:
            n = lib.axon_stop_nrt_profile(str(output_dir).encode())
            if n < 0:
                raise RuntimeError(f"axon_stop_nrt_profile rc={n}")
            if n == 0:
                print("WARNING: NTFF capture wrote 0 files")

    mod = types.ModuleType("antenv.axon_hooks")
    mod.get_axon_ntff_profile_hook = lambda: _hook
    mod.set_axon_ntff_profile_hook = lambda h: None
    sys.modules["antenv.axon_hooks"] = mod


def run_traced(x2, x3, trace_cores=None, **kw):
    _ensure_ntff_hook()
    params = prepare_params(**kw)
    nc = _get_program()
    in_maps = make_in_maps(x2, x3, params)
    res = run_bass_kernel_spmd(nc, in_maps, list(range(NCORES)),
                               trace=True, trace_cores=trace_cores)
    out = np.stack([np.asarray(res.results[b]["out"]).astype(np.float32)
                    .reshape(C, H, W) for b in range(NCORES)])
    return out, res


# revision 27
# speedup vs baseline: 1.0063x; 1.0027x over previous
"""Trainium2 Bass kernel for nn_KTM_71339406786898 (optimized v3).

Fused dual-input attention block (see reference):
  q = wq@(x2+x3)+bq, k = wk@(x2*x3)+bk           (CQ=16 channels)
  energy[i,j] = q[:,i].k[:,j];  attn = softmax_j
  out{2,3} = v{2,3} @ attn^T;  z{2,3} = gamma*out + x
  h{2,3} = relu(BN(conv3x3(z)));  out = wo@(w2_1@h2 + w3_1@h3 ...)+bo

Sharding: data-parallel over batch B=8 across 8 NeuronCores.

v3 design notes:
 * exp split across ScalarE (true exp -> bf16) and VectorE (Schraudolph:
   i16 = trunc(x*128/ln2 + (16256-6.85)) bitcast bf16; zero-mean ~2%
   sawtooth that cancels under softmax).  Granules of 2 j-tiles
   alternate ACT/DVE so both engines stream concurrently.
 * all parameters packed into two DRAM blobs (one f32r, one bf16) and x
   pre-stacked host-side -> 4 DMA issues total in setup (dma_start issue
   costs ~0.6us each on a sequencer, so issue count matters).
 * q projection contracts [x2;x3;ones] with duplicated wq rows (no xsum
   tensor); v bias rides on the residual tile xr = x + gamma*bv since
   softmax rows sum to 1; v projections are 2-way row-tiled K=32.
 * normalization is software-pipelined into the next chunk with
   accp bufs=2: d-copy + reciprocal_approx_fast + gpsimd broadcast +
   (acc * r) + residual-add, all off the chunk-critical path.
 * conv3x3 in bf16 (tap stacks via 8 sb->sb DMAs/chunk), relu+BN-bias on
   ScalarE, final fused 1x1 bias via a ones-row, output bounce on ACT.
 * PSUM: 4 banks energy double-buffer + 2 acc + 1 conv = 7.
"""

import sys

import ml_dtypes
import numpy as np

for _p in ("/opt/trn_rl_repo", "/root/.axon_site/_ro/trn_rl_repo"):
    if _p not in sys.path:
        sys.path.append(_p)

import concourse.bass as bass
import concourse.mybir as mybir
import concourse.tile as tile
from concourse import bacc
from concourse.bass_utils import run_bass_kernel_spmd

B, C, H, W = 8, 32, 64, 64
CQ = C // 2
HW = H * W
NCORES = 8

IC = 512            # i-chunk (attention query columns per chunk)
NCH = HW // IC      # 8 chunks
JT = 128            # j-tile (attention key rows per tile = partitions)
NJT = HW // JT      # 32 j-tiles
G = 2               # j-tiles per exp granule
NG = NJT // G       # 16 granules per chunk
XSP = 576           # exp column split: ACT does [0:XSP], DVE [XSP:2*IC]
PW = W + 2          # padded conv width (66)
PHW = PW * (H + 2)  # padded conv plane (66*66)
RPC = IC // W       # spatial rows per chunk (8)
SEG = RPC * PW + W  # stack copy length per chunk (592)

A16 = 184.66496     # 128/ln2
B16 = 16249.15      # 16256 - 6.85 (zero-mean log-ratio calibration)

F32 = mybir.dt.float32
F32R = mybir.dt.float32r
BF16 = mybir.dt.bfloat16
I16 = mybir.dt.int16
AF = mybir.ActivationFunctionType
ALU = mybir.AluOpType

# const pack layouts (free-dim column ranges)
PF_COLS = 66    # f32 pack: wq65[0:16] wk33[16:32] wv23[32:64] brelu[64] c23[65]
PB_COLS = 192   # bf16 pack: w2A w2B w3A w3B w23c wab65 (6 x 32)


def build_program():
    nc = bacc.Bacc("TRN2", target_bir_lowering=False, debug=False)

    xbothd = nc.dram_tensor("xboth", [2 * C + 1, HW], F32R,
                            kind="ExternalInput").ap()
    pfd = nc.dram_tensor("pf32", [128, PF_COLS], F32R, kind="ExternalInput").ap()
    pbd = nc.dram_tensor("pb16", [128, PB_COLS], BF16, kind="ExternalInput").ap()
    outd = nc.dram_tensor("out", [C, HW], BF16, kind="ExternalOutput").ap()

    with tile.TileContext(nc) as tc:
        _emit(nc, tc, xbothd, pfd, pbd, outd)
    nc.compile()
    return nc


def _emit(nc, tc, xbothd, pfd, pbd, outd):
    from contextlib import ExitStack

    ctx = ExitStack()
    with ctx:
        consts = ctx.enter_context(tc.tile_pool(name="consts", bufs=1))
        xp = ctx.enter_context(tc.tile_pool(name="xp", bufs=1))
        qk = ctx.enter_context(tc.tile_pool(name="qk", bufs=1))
        vs = ctx.enter_context(tc.tile_pool(name="vs", bufs=1))
        zpool = ctx.enter_context(tc.tile_pool(name="zpool", bufs=1))
        stk = ctx.enter_context(tc.tile_pool(name="stk", bufs=1))
        es = ctx.enter_context(tc.tile_pool(name="es", bufs=3))
        norm = ctx.enter_context(tc.tile_pool(name="norm", bufs=2))
        rsp = ctx.enter_context(tc.tile_pool(name="rsp", bufs=1))
        obp = ctx.enter_context(tc.tile_pool(name="obp", bufs=2))
        psum = ctx.enter_context(tc.tile_pool(name="psum", bufs=2, space="PSUM"))
        wpsp = ctx.enter_context(tc.tile_pool(name="wpsp", bufs=1, space="PSUM"))
        accp = ctx.enter_context(tc.tile_pool(name="accp", bufs=2, space="PSUM"))
        convp = ctx.enter_context(tc.tile_pool(name="convp", bufs=1, space="PSUM"))

        # ---- setup DMA issues across 3 queues ----
        xa = xp.tile([2 * C + 1, HW], F32R, tag="xa")
        nc.sync.dma_start(out=xa[:], in_=xbothd)
        pf = consts.tile([128, PF_COLS], F32R, tag="pf")
        nc.scalar.dma_start(out=pf[:], in_=pfd)
        pb = consts.tile([128, PB_COLS], BF16, tag="pb")
        nc.gpsimd.dma_start(out=pb[:], in_=pbd)
        x3c = xp.tile([C, HW], F32R, tag="x3c")
        nc.sync.dma_start(out=x3c[:], in_=xbothd[C:2 * C, :])

        wq65 = pf[0:2 * C + 1, 0:CQ]
        wk33 = pf[0:C + 1, CQ:2 * CQ]
        wv23 = pf[0:2 * C, 2 * CQ:2 * CQ + C]
        brelu = pf.bitcast(F32)[0:2 * C, 64:65]
        c23 = pf.bitcast(F32)[0:2 * C, 65:66]
        w2A = pb[:, 0:32]
        w2B = pb[:, 32:64]
        w3A = pb[:, 64:96]
        w3B = pb[:, 96:128]
        w23c = pb[0:2 * C, 128:160]
        wab65 = pb[0:2 * C + 1, 160:192]

        wrm = xp.tile([JT, IC], BF16, tag="wrm")
        nc.vector.memset(wrm[:], 0.25)
        xmul = xp.tile([C + 1, HW], F32R, tag="xmul")
        xr = xp.tile([2 * C, HW], BF16, tag="xr")

        # ---- conv z planes + tap stacks (bf16) ----
        zp = zpool.tile([2 * C, PHW], BF16, tag="zp")
        nc.gpsimd.memset(zp[:], 0.0)
        zp3 = zp.rearrange("p (h w) -> p h w", h=H + 2, w=PW)
        stkA2 = stk.tile([JT, PHW], BF16, tag="stkA2")
        stkB2 = stk.tile([JT, PHW], BF16, tag="stkB2")
        stkA3 = stk.tile([JT, PHW], BF16, tag="stkA3")
        stkB3 = stk.tile([JT, PHW], BF16, tag="stkB3")
        s3 = {nm: t.rearrange("p (h w) -> p h w", h=H + 2, w=PW)
              for nm, t in (("A2", stkA2), ("B2", stkB2),
                            ("A3", stkA3), ("B3", stkB3))}

        vstack = vs.tile([JT, NJT, 2 * C + 1], BF16, tag="vstack")
        nc.gpsimd.memset(vstack[:, :, 2 * C:2 * C + 1], 1.0)

        # ---- q projection (ACT casts; replicas for 4-way row tiling) ----
        q_sb = qk.tile([112, HW], BF16, tag="q")
        k_sb = qk.tile([112, HW], BF16, tag="k")
        QKC = 2 * IC
        for ci in range(4):
            off = ci * QKC
            p = psum.tile([JT, QKC], F32, tag="big", name=f"qp{ci}")
            for s in range(0, QKC, IC):
                nc.tensor.matmul(
                    p[0:CQ, s:s + IC], wq65,
                    xa[:, off + s:off + s + IC], start=True, stop=True)
            if ci < 2:
                nc.scalar.copy(out=q_sb[0:CQ, off:off + QKC], in_=p[0:CQ, :])
            else:
                nc.vector.tensor_copy(out=q_sb[0:CQ, off:off + QKC],
                                      in_=p[0:CQ, :])
            for rg in (1, 2, 3):
                nc.sync.dma_start(out=q_sb[32 * rg:32 * rg + CQ, off:off + QKC],
                                  in_=q_sb[0:CQ, off:off + QKC])

        # ---- xmul = x2*x3 (DVE + gpsimd split), ones row via DMA ----
        XSPL = 2816
        nc.vector.tensor_mul(xmul[0:C, 0:XSPL], xa[0:C, 0:XSPL],
                             x3c[0:C, 0:XSPL])
        nc.gpsimd.tensor_tensor(xmul[0:C, XSPL:HW], xa[0:C, XSPL:HW],
                                x3c[0:C, XSPL:HW], op=ALU.mult)
        nc.scalar.dma_start(out=xmul[C:C + 1, :], in_=xa[2 * C:2 * C + 1, :])

        # ---- xr = x + gamma*bv (residual with v-bias folded in), bf16 ----
        nc.scalar.activation(xr[:], xa[0:2 * C, :], AF.Identity,
                             bias=c23)

        # ---- k projection (DVE casts) ----
        for ci in range(4):
            off = ci * QKC
            p = psum.tile([JT, QKC], F32, tag="big", name=f"kp{ci}")
            for s in range(0, QKC, IC):
                nc.tensor.matmul(
                    p[0:CQ, s:s + IC], wk33,
                    xmul[:, off + s:off + s + IC], start=True, stop=True)
            nc.vector.tensor_copy(out=k_sb[0:CQ, off:off + QKC], in_=p[0:CQ, :])
            for rg in (1, 2, 3):
                nc.scalar.dma_start(out=k_sb[32 * rg:32 * rg + CQ, off:off + QKC],
                                    in_=k_sb[0:CQ, off:off + QKC])

        # ---- v projections: 2-way row-tiled K=32, 4 j-tiles per batch
        # (before k: does not depend on xmul, keeps the PE gap-free) ----
        for b4 in range(NJT // 4):
            vp = psum.tile([JT, 4 * 2 * C], F32, tag="big", name=f"vp{b4}")
            for t in range(4):
                jt = 4 * b4 + t
                js = slice(jt * JT, (jt + 1) * JT)
                nc.tensor.matmul(
                    vp[:, t * 2 * C:t * 2 * C + C],
                    xa[0:C, js], wv23[0:C, :],
                    start=True, stop=True, tile_position=(0, 0))
                nc.tensor.matmul(
                    vp[:, t * 2 * C + C:(t + 1) * 2 * C],
                    xa[C:2 * C, js], wv23[C:2 * C, :],
                    start=True, stop=True, tile_position=(32, 0))
            nc.vector.tensor_copy(
                out=vstack[:, 4 * b4:4 * b4 + 4, 0:2 * C], in_=vp[:])

        # ---- second warm-up burst, forced (via the wrm touch below,
        # which depends on the DVE cast queue) to run after the v phase:
        # v's per-tile weight loads break the HAM busy detector, so the PE
        # must be re-warmed right before the main loop ----
        nc.vector.memset(wrm[0:1, 0:1], 0.25)
        wps2 = wpsp.tile([JT, IC], F32, tag="wps", name="wps2")
        for i in range(20):
            nc.tensor.matmul(wps2[:], wrm[:, 0:JT], wrm[:],
                             start=True, stop=True)

        # rstk double buffers with preset ones row (final 1x1 bias)
        rstkA = rsp.tile([2 * C + 1, IC], BF16, tag="rstkA")
        nc.vector.memset(rstkA[2 * C:2 * C + 1, :], 1.0)
        rstkB = rsp.tile([2 * C + 1, IC], BF16, tag="rstkB")
        nc.vector.memset(rstkB[2 * C:2 * C + 1, :], 1.0)

        st = {}
        st[0, "rstk"] = rstkA
        st[1, "rstk"] = rstkB

        def emit_recip(ic):
            acc = st[ic, "acc"]
            d_t = norm.tile([1, IC], F32, tag="d", name=f"d{ic}")
            nc.vector.tensor_copy(out=d_t[:], in_=acc[2 * C:2 * C + 1, :])
            r_t = norm.tile([1, IC], F32, tag="r", name=f"r{ic}")
            nc.vector.reciprocal_approx_fast(out=r_t[:], in_=d_t[:])
            st[ic, "r"] = r_t

        def emit_bcast(ic):
            rbc = norm.tile([2 * C, IC], F32, tag="rbc", name=f"rbc{ic}")
            nc.gpsimd.partition_broadcast(rbc[:], st[ic, "r"][:])
            st[ic, "rbc"] = rbc

        def emit_mul(ic):
            zt = norm.tile([2 * C, IC], BF16, tag="zt", name=f"zt{ic}")
            nc.vector.tensor_mul(zt[:], st[ic, "acc"][0:2 * C, :],
                                 st[ic, "rbc"][:])
            st[ic, "zt"] = zt

        def emit_add(ic):
            r0 = RPC * ic
            i0 = ic * IC
            nc.vector.tensor_add(
                zp3[:, 1 + r0:1 + r0 + RPC, 1:1 + W],
                st[ic, "zt"][:].rearrange("p (a b) -> p a b", a=RPC, b=W),
                xr[:, i0:i0 + IC].rearrange("p (a b) -> p a b", a=RPC, b=W),
            )

        def emit_stacks(n, half):
            p0 = PW * RPC * n
            ln = min(SEG, PHW - p0 - 2 * PW - 2)
            r0s, stA, stB = ((0, stkA2, stkB2), (C, stkA3, stkB3))[half]
            for a in range(4):
                offA = (a // 3) * PW + (a % 3)
                nc.sync.dma_start(
                    out=stA[32 * a:32 * a + C, p0:p0 + ln],
                    in_=zp[r0s:r0s + C, p0 + offA:p0 + offA + ln])
                tb = a + 4
                offB = (tb // 3) * PW + (tb % 3)
                nc.gpsimd.dma_start(
                    out=stB[32 * a:32 * a + C, p0:p0 + ln],
                    in_=zp[r0s:r0s + C, p0 + offB:p0 + offB + ln])

        def emit_conv(n):
            r0 = RPC * n
            cp = convp.tile([2 * C, IC], F32, tag="cv", name=f"cp{n}")
            nc.tensor.matmul(cp[0:C, :], w2A, s3["A2"][:, r0:r0 + RPC, 0:W],
                             start=True, stop=False)
            nc.tensor.matmul(cp[0:C, :], w2B, s3["B2"][:, r0:r0 + RPC, 0:W],
                             start=False, stop=False)
            nc.tensor.matmul(cp[0:C, :], w23c[0:C, :],
                             zp3[0:C, 2 + r0:2 + r0 + RPC, 2:2 + W],
                             start=False, stop=True)
            nc.tensor.matmul(cp[C:2 * C, :], w3A,
                             s3["A3"][:, r0:r0 + RPC, 0:W],
                             start=True, stop=False, tile_position=(0, 32))
            nc.tensor.matmul(cp[C:2 * C, :], w3B,
                             s3["B3"][:, r0:r0 + RPC, 0:W],
                             start=False, stop=False, tile_position=(0, 32))
            nc.tensor.matmul(cp[C:2 * C, :], w23c[C:2 * C, :],
                             zp3[C:2 * C, 2 + r0:2 + r0 + RPC, 2:2 + W],
                             start=False, stop=True, tile_position=(32, 32))
            st[n, "cp"] = cp

        def emit_rstk(n):
            rs = st[n % 2, "rstk"]
            nc.scalar.activation(rs[0:2 * C, :], st[n, "cp"][:], AF.Relu,
                                 bias=brelu)

        def emit_wab(n):
            op = convp.tile([2 * C, IC], F32, tag="cv", name=f"op{n}")
            nc.tensor.matmul(op[0:C, :], wab65, st[n % 2, "rstk"][:],
                             start=True, stop=True)
            st[n, "op"] = op

        def emit_ob(n):
            ob = obp.tile([C, IC], BF16, tag="ob", name=f"ob{n}")
            nc.scalar.copy(out=ob[:], in_=st[n, "op"][0:C, :])
            nc.sync.dma_start(out=outd[:, n * IC:(n + 1) * IC], in_=ob[:])

        def emit_E(ic, g):
            i0 = ic * IC
            e_ps = psum.tile([JT, G * IC], F32, tag="big", name=f"eps{ic}_{g}")
            for t in range(G):
                jt = G * g + t
                rt = jt % 4
                nc.tensor.matmul(
                    e_ps[:, t * IC:(t + 1) * IC],
                    k_sb[32 * rt:32 * rt + CQ, jt * JT:(jt + 1) * JT],
                    q_sb[32 * rt:32 * rt + CQ, i0:i0 + IC],
                    start=True, stop=True, tile_position=(32 * rt, 0))
            st[ic, g, "eps"] = e_ps

        def emit_exp(ic, g):
            e_ps = st.pop((ic, g, "eps"))
            e_sb = es.tile([JT, G * IC], BF16, tag="esb", name=f"esb{ic}_{g}")
            nc.scalar.activation(e_sb[:, 0:XSP], e_ps[:, 0:XSP], AF.Exp)
            nc.vector.tensor_scalar(
                out=e_sb.bitcast(I16)[:, XSP:G * IC],
                in0=e_ps[:, XSP:G * IC],
                scalar1=A16, scalar2=B16, op0=ALU.mult, op1=ALU.add)
            st[ic, g, "esb"] = e_sb

        def emit_AV(ic, g):
            if (ic, "acc") not in st:
                st[ic, "acc"] = accp.tile([2 * C + 1, IC], F32, tag="acc",
                                          name=f"acc{ic}")
            acc = st[ic, "acc"]
            e_sb = st.pop((ic, g, "esb"))
            for t in range(G):
                jt = G * g + t
                nc.tensor.matmul(
                    acc[:], vstack[:, jt, :], e_sb[:, t * IC:(t + 1) * IC],
                    start=(jt == 0), stop=(jt == NJT - 1))

        # ---- PE warm-up burst: dense full-array matmuls with one
        # stationary operand flip HAM to K=8/8 (2.4 GHz) ----
        wps = wpsp.tile([JT, IC], F32, tag="wps")
        for i in range(24):
            nc.tensor.matmul(wps[:], wrm[:, 0:JT], wrm[:],
                             start=True, stop=True)

        # ---- main loop: 2-chunk software pipeline over 16 granules ----
        NTOT = NCH * NG
        emit_E(0, 0)
        for gi in range(NTOT):
            ic, g = divmod(gi, NG)
            if gi + 1 < NTOT:
                ic2, g2 = divmod(gi + 1, NG)
                emit_E(ic2, g2)
            emit_exp(ic, g)
            emit_AV(ic, g)
            if g == 0 and ic >= 1:
                emit_recip(ic - 1)
            elif g == 1 and ic >= 1:
                emit_bcast(ic - 1)
            elif g == 2 and ic >= 1:
                emit_mul(ic - 1)
            elif g == 3 and ic >= 1:
                emit_add(ic - 1)
            elif g == 4 and ic >= 2:
                emit_stacks(ic - 2, 0)
            elif g == 5 and ic >= 2:
                emit_stacks(ic - 2, 1)
            elif g == 6 and ic >= 2:
                emit_conv(ic - 2)
            elif g == 8 and ic >= 2:
                emit_rstk(ic - 2)
            elif g == 10 and ic >= 2:
                emit_wab(ic - 2)
            elif g == 12 and ic >= 2:
                emit_ob(ic - 2)

        # ---- drain ----
        ic = NCH - 1
        emit_recip(ic)
        emit_bcast(ic)
        emit_mul(ic)
        emit_add(ic)
        for n in (NCH - 2, NCH - 1):
            emit_stacks(n, 0)
            emit_stacks(n, 1)
            emit_conv(n)
            emit_rstk(n)
            emit_wab(n)
            emit_ob(n)


def prepare_params(wq, bq, wk, bk, wv2, bv2, wv3, bv3, gamma2, gamma3,
                   w2_3, bn2_s, bn2_b, w2_1, b2_1,
                   w3_3, bn3_s, bn3_b, w3_1, b3_1, wo, bo):
    f = np.float32
    bf = ml_dtypes.bfloat16
    wq, bq, wk, bk = (np.asarray(a, f) for a in (wq, bq, wk, bk))
    wv2, bv2, wv3, bv3 = (np.asarray(a, f) for a in (wv2, bv2, wv3, bv3))
    w2_3, bn2_s, bn2_b = (np.asarray(a, f) for a in (w2_3, bn2_s, bn2_b))
    w3_3, bn3_s, bn3_b = (np.asarray(a, f) for a in (w3_3, bn3_s, bn3_b))
    w2_1, b2_1, w3_1, b3_1 = (np.asarray(a, f) for a in (w2_1, b2_1, w3_1, b3_1))
    wo, bo = np.asarray(wo, f), np.asarray(bo, f)
    g2 = f(np.asarray(gamma2).reshape(-1)[0])
    g3 = f(np.asarray(gamma3).reshape(-1)[0])

    pf32 = np.zeros((128, PF_COLS), f)
    pf32[0:C, 0:CQ] = wq.T
    pf32[C:2 * C, 0:CQ] = wq.T
    pf32[2 * C, 0:CQ] = bq
    pf32[0:C, CQ:2 * CQ] = wk.T
    pf32[C, CQ:2 * CQ] = bk
    pf32[0:C, 2 * CQ:2 * CQ + C] = wv2.T * g2
    pf32[C:2 * C, 2 * CQ:2 * CQ + C] = wv3.T * g3
    pf32[0:C, 64] = bn2_b
    pf32[C:2 * C, 64] = bn3_b
    pf32[0:C, 65] = g2 * bv2
    pf32[C:2 * C, 65] = g3 * bv3

    def conv_stacks(w3x3, bn_s):
        ws = w3x3 * bn_s[:, None, None, None]
        Am = np.zeros((4 * C, C), f)
        Bm = np.zeros((4 * C, C), f)
        for a in range(4):
            Am[32 * a:32 * a + C] = ws[:, :, a // 3, a % 3].T
            tb = a + 4
            Bm[32 * a:32 * a + C] = ws[:, :, tb // 3, tb % 3].T
        return Am, Bm, ws[:, :, 2, 2].T.copy()

    w2A, w2B, w2c = conv_stacks(w2_3, bn2_s)
    w3A, w3B, w3c = conv_stacks(w3_3, bn3_s)

    pb16 = np.zeros((128, PB_COLS), f)
    pb16[:, 0:32] = w2A
    pb16[:, 32:64] = w2B
    pb16[:, 64:96] = w3A
    pb16[:, 96:128] = w3B
    pb16[0:C, 128:160] = w2c
    pb16[C:2 * C, 128:160] = w3c
    pb16[0:C, 160:192] = (wo @ w2_1).T
    pb16[C:2 * C, 160:192] = (wo @ w3_1).T
    pb16[2 * C, 160:192] = wo @ (b2_1 + b3_1) + bo

    return {"pf32": pf32, "pb16": pb16.astype(bf)}


_CACHED = {}


def _get_program():
    if "nc" not in _CACHED:
        _CACHED["nc"] = build_program()
    return _CACHED["nc"]


def make_in_maps(x2, x3, params):
    x2 = np.asarray(x2, np.float32).reshape(B, C, HW)
    x3 = np.asarray(x3, np.float32).reshape(B, C, HW)
    ones = np.ones((1, HW), np.float32)
    return [
        {"xboth": np.concatenate([x2[b], x3[b], ones], 0), **params}
        for b in range(NCORES)
    ]


def kernel(x2, x3, **kw):
    params = prepare_params(**kw)
    nc = _get_program()
    in_maps = make_in_maps(x2, x3, params)
    res = run_bass_kernel_spmd(nc, in_maps, list(range(NCORES)))
    out = np.stack([np.asarray(res.results[b]["out"]).astype(np.float32)
                    .reshape(C, H, W) for b in range(NCORES)])
    return out


def _ensure_ntff_hook():
    """Register the ctypes NTFF profile hook (agent image lacks axon_hooks)."""
    import contextlib
    import ctypes
    import types

    if "antenv.axon_hooks" in sys.modules:
        return
    so_path = "/opt/axon/libaxon_pjrt.so"
    lib = ctypes.CDLL(so_path)
    lib.axon_start_nrt_profile.argtypes = [
        ctypes.POINTER(ctypes.c_int64), ctypes.c_size_t]
    lib.axon_start_nrt_profile.restype = ctypes.c_int64
    lib.axon_stop_nrt_profile.argtypes = [ctypes.c_char_p]
    lib.axon_stop_nrt_profile.restype = ctypes.c_int64

    @contextlib.contextmanager
    def _hook(output_dir, device_ids):
        import jax
        jax.devices()
        if device_ids:
            ids = (ctypes.c_int64 * len(device_ids))(*device_ids)
            rc = lib.axon_start_nrt_profile(ids, len(device_ids))
        else:
            rc = lib.axon_start_nrt_profile(None, 0)
        if rc != 0:
            raise RuntimeError(f"axon_start_nrt_profile rc={rc}")
        try:
            yield
        finally:
            n = lib.axon_stop_nrt_profile(str(output_dir).encode())
            if n < 0:
                raise RuntimeError(f"axon_stop_nrt_profile rc={n}")
            if n == 0:
                print("WARNING: NTFF capture wrote 0 files")

    mod = types.ModuleType("antenv.axon_hooks")
    mod.get_axon_ntff_profile_hook = lambda: _hook
    mod.set_axon_ntff_profile_hook = lambda h: None
    sys.modules["antenv.axon_hooks"] = mod


def run_traced(x2, x3, trace_cores=None, **kw):
    _ensure_ntff_hook()
    params = prepare_params(**kw)
    nc = _get_program()
    in_maps = make_in_maps(x2, x3, params)
    res = run_bass_kernel_spmd(nc, in_maps, list(range(NCORES)),
                               trace=True, trace_cores=trace_cores)
    out = np.stack([np.asarray(res.results[b]["out"]).astype(np.float32)
                    .reshape(C, H, W) for b in range(NCORES)])
    return out, res
